# revision 1
# baseline (speedup 1.0000x reference)
"""Trainium2 Bass kernel for the ragged per-layer decoder stack.

out[b, i, a] = sum_{j<=i} sum_f x[b, j, f] * W[i, j, f, a]
  x: [256, 12, 2048] f32,  W: [12, 12, 2048, 768] f32 -> out: [256, 12, 768] f32

Sharding: W's d_features axis (F=2048) is split across the 8 NeuronCores
(256 features each). Each core contracts its feature slice against the
lower-triangular (j<=i) weight blocks and produces a full partial output
[12, 256, 768]; the host sums the 8 partials (the all-reduce) and
transposes back to [256, 12, 768].

Matmuls run in bf16 (hostside cast) with fp32 PSUM accumulation.
Weight DMAs are j-merged per (i, k-tile) and partition-major packed so
every partition row is one long contiguous run (>=4KB descriptors
saturate the HBM bus: measured 345 GB/s). Output DMAs go out on the ACT
HWDGE ring so their semaphore waits cannot head-of-line-block the W
stream on the SP ring. PSUM accumulation runs k-major so each group can
start as soon as its first k-block lands.
"""

import numpy as np
import ml_dtypes

import concourse.bass as bass
import concourse.tile as tile
from concourse import bacc, mybir
from concourse.bass_utils import run_bass_kernel_spmd

BF16 = ml_dtypes.bfloat16

# Problem shape (hardcoded per contract)
B = 256      # batch
L = 12       # layers
F = 2048     # d_features
A = 768      # d_activations
NCORES = 8
FC = F // NCORES      # feature slice per core = 256
P = 128               # partitions
NK = FC // P          # k-tiles per core slice = 2
NB = B // P           # batch tiles = 2
AC = 384              # activation chunk per matmul (2 chunks of 384 <= 512 PSUM)
NPAIR = sum(i + 1 for i in range(L)) * NK   # 156 weight tiles per core

_PAIRS = [(i, j) for i in range(L) for j in range(i + 1)]

# --- tuning knobs (affect build_module; set before first call) ---
WBUFS = 6         # W block pool slots (each sized [128, 12*768] bf16)
OBUFS = 4         # output tile pool slots
PSBUFS = 8        # PSUM pool slots (banks)
COPY_SPLIT = False  # alternate PSUM->SBUF copies between DVE and ACT
SKIP_MM = False     # diagnostic: drop matmuls+copies (DMA-only span)
SKIP_OUT = False    # diagnostic: drop copies + out-DMA
HWLOOP = True       # use tc.For_i for repeat>1 (bench only)
OBF16 = True        # write partial outputs as bf16 (host sums in fp32)
I_DESC = False      # process i in descending order (ascending measured faster)
KMAJOR = True       # accumulate k-major (j inner) so group starts on block k0
INTERLEAVE_AC = False  # interleave ac0/ac1 MMs sharing the stationary lhsT
ACSPLIT = False     # split A as 512+256 instead of 384+384
PSUM_DMA = False    # (unsupported: bass rejects DMA from PSUM)
PE_ONLY = False     # diagnostic: preload W for i<=IMAX once; loop MMs only
IMAX = L            # limit i range (diagnostics)

# W block (i, k) tile offset in wpack: tiles [j=0..i] for fixed k
_WBASE = {}
_off = 0
for _i in range(L):
    for _k in range(NK):
        _WBASE[(_i, _k)] = _off
        _off += _i + 1
assert _off == NPAIR


def _emit_kernel(ctx, tc, xpack, wpack, out, repeat=1):
    nc = tc.nc
    xpool = ctx.enter_context(tc.tile_pool(name="xpool", bufs=1))
    wpool = ctx.enter_context(tc.tile_pool(name="wpool", bufs=WBUFS))
    opool = ctx.enter_context(tc.tile_pool(name="opool", bufs=OBUFS))
    pspool = ctx.enter_context(tc.tile_pool(name="pspool", bufs=PSBUFS, space="PSUM"))

    # x resident in SBUF for the whole kernel, one tile per k-slice:
    # xts[k][p, j*B + b] = x[b, j, c*FC + k*P + p]
    xts = []
    for k in range(NK):
        xt = xpool.tile([P, L * B], mybir.dt.bfloat16, tag=f"x{k}")
        nc.sync.dma_start(xt[:], xpack[:, k * L * B:(k + 1) * L * B])
        xts.append(xt)

    preloaded = None
    if PE_ONLY:
        # preload all W blocks for i < IMAX once; loop body has no W DMAs
        preloaded = {}
        for i in range(IMAX):
            n = i + 1
            for k in range(NK):
                wt = wpool.tile([P, n * A], mybir.dt.bfloat16,
                                name=f"wpre{i}_{k}", tag=f"wpre{i}_{k}", bufs=1)
                base = _WBASE[(i, k)] * A
                nc.sync.dma_start(wt[:], wpack[:, base:base + n * A])
                preloaded[(i, k)] = wt

    if repeat > 1 and HWLOOP:
        with tc.For_i(0, repeat, 1, hint_engines=(
                mybir.EngineType.PE, mybir.EngineType.SP)):
            _emit_body(tc, xts, wpack, out, wpool, opool, pspool, preloaded)
    else:
        for _ in range(repeat):
            _emit_body(tc, xts, wpack, out, wpool, opool, pspool, preloaded)


def _emit_body(tc, xts, wpack, out, wpool, opool, pspool, preloaded=None):
    nc = tc.nc
    iorder = range(L - 1, -1, -1) if I_DESC else range(L)
    for i in iorder:
        if i >= IMAX:
            continue
        n = i + 1
        # j-merged weight blocks, one per k-tile: [128, n*768] bf16.
        # wpack is partition-major, so each partition row is one
        # contiguous n*1536B run (>=4KB descriptors saturate the bus).
        wts = []
        for k in range(NK):
            if preloaded is not None:
                wts.append(preloaded[(i, k)])
                continue
            wt = wpool.tile([P, n * A], mybir.dt.bfloat16, tag="w")
            base = _WBASE[(i, k)] * A
            nc.sync.dma_start(wt[:], wpack[:, base:base + n * A])
            wts.append(wt)
        if KMAJOR:
            jks = [(j, k) for k in range(NK) for j in range(n)]
        else:
            jks = [(j, k) for j in range(n) for k in range(NK)]
        acs = [(0, 512), (512, 256)] if ACSPLIT else [(0, AC), (AC, AC)]
        for bt in range(NB):
            if SKIP_MM:
                continue
            pss = [pspool.tile([P, w], mybir.dt.float32, name=f"ps{ci}",
                               tag=f"ps{ci}", bufs=PSBUFS // 2)
                   for ci, (_, w) in enumerate(acs)]
            if INTERLEAVE_AC:
                for t, (j, k) in enumerate(jks):
                    lhsT = xts[k][:, j * B + bt * P:j * B + bt * P + P]
                    for ps, (off, w) in zip(pss, acs):
                        nc.tensor.matmul(
                            ps[:], lhsT,
                            wts[k][:, j * A + off:j * A + off + w],
                            start=(t == 0), stop=(t == len(jks) - 1),
                            skip_group_check=True,
                        )
            else:
                for ps, (off, w) in zip(pss, acs):
                    for t, (j, k) in enumerate(jks):
                        nc.tensor.matmul(
                            ps[:],
                            xts[k][:, j * B + bt * P:j * B + bt * P + P],
                            wts[k][:, j * A + off:j * A + off + w],
                            start=(t == 0), stop=(t == len(jks) - 1),
                        )
            if SKIP_OUT:
                continue
            if PSUM_DMA:
                for ps, (off, w) in zip(pss, acs):
                    nc.scalar.dma_start(
                        out[i, bt * P:(bt + 1) * P, off:off + w], ps[:])
                continue
            odt = mybir.dt.bfloat16 if OBF16 else mybir.dt.float32
            ot = opool.tile([P, A], odt)
            if COPY_SPLIT:
                nc.vector.tensor_copy(ot[:, 0:acs[0][1]], pss[0][:])
                nc.scalar.copy(ot[:, acs[0][1]:A], pss[1][:])
            else:
                nc.vector.tensor_copy(ot[:, 0:acs[0][1]], pss[0][:])
                nc.vector.tensor_copy(ot[:, acs[0][1]:A], pss[1][:])
            # out-DMA on the ACT HWDGE ring: its wait on the copy sem must
            # not head-of-line-block the W stream on the SP ring.
            nc.scalar.dma_start(out[i, bt * P:(bt + 1) * P, :], ot[:])


_NC_CACHE = {}


def build_module(repeat=1):
    key = (repeat, WBUFS, OBUFS, PSBUFS, COPY_SPLIT, SKIP_MM, SKIP_OUT,
           HWLOOP, OBF16, I_DESC, KMAJOR, INTERLEAVE_AC, ACSPLIT, PSUM_DMA,
           PE_ONLY, IMAX)
    if key in _NC_CACHE:
        return _NC_CACHE[key]
    from contextlib import ExitStack
    nc = bacc.Bacc(
        "TRN2",
        target_bir_lowering=False,
        debug=False,
        enable_asserts=False,
        num_devices=NCORES,
    )
    xpack = nc.dram_tensor(
        "xpack", [P, NK * L * B], mybir.dt.bfloat16, kind="ExternalInput").ap()
    wpack = nc.dram_tensor(
        "wpack", [P, NPAIR * A], mybir.dt.bfloat16, kind="ExternalInput").ap()
    out = nc.dram_tensor(
        "out", [L, B, A],
        mybir.dt.bfloat16 if (OBF16 and not PSUM_DMA) else mybir.dt.float32,
        kind="ExternalOutput").ap()
    with tile.TileContext(nc) as tc:
        with ExitStack() as ctx:
            _emit_kernel(ctx, tc, xpack, wpack, out, repeat=repeat)
    nc.compile()
    _NC_CACHE[key] = nc
    return nc


def prep_inputs(x, W):
    """Build per-core packed inputs. Returns (xpacks[8], wpacks[8])."""
    # xpack[c][p, (k*L + j)*B + b] = x[b, j, c*FC + k*P + p]
    xb = np.asarray(x, dtype=BF16)                       # [256, 12, 2048]
    xr = xb.reshape(B, L, NCORES, NK, P).transpose(2, 4, 3, 1, 0)
    xpacks = np.ascontiguousarray(xr).reshape(NCORES, P, NK * L * B)

    # wpack[c]: partition-major; per (i, k) block occupies free columns
    # [_WBASE*A : (_WBASE+n)*A], j inner:
    #   wpack[c][p, (_WBASE[(i,k)] + j)*A + a] = W[i, j, c*FC + k*P + p, a]
    Ii = [i for i, j in _PAIRS]
    Jj = [j for i, j in _PAIRS]
    Wtri = np.asarray(W, dtype=BF16)[Ii, Jj]             # [78, 2048, 768]
    Wtri = Wtri.reshape(len(_PAIRS), NCORES, NK, P, A)   # [78, c, k, p, a]
    pidx = {}
    for t, (i, j) in enumerate(_PAIRS):
        pidx[(i, j)] = t
    sel_pair, sel_k = [], []
    for i in range(L):
        for k in range(NK):
            for j in range(i + 1):
                sel_pair.append(pidx[(i, j)])
                sel_k.append(k)
    Wp = Wtri[sel_pair, :, sel_k]                        # [156, c, 128, 768]
    Wp = np.ascontiguousarray(Wp.transpose(1, 2, 0, 3))  # [c, p, 156, a]
    wpacks = Wp.reshape(NCORES, P, NPAIR * A)
    return xpacks, wpacks


def run(x, W, trace=False, **kw):
    """Run the SPMD kernel; returns (full_output, BassKernelResults)."""
    x = np.asarray(x, dtype=np.float32)
    W = np.asarray(W, dtype=np.float32)
    xpacks, wpacks = prep_inputs(x, W)
    nc = build_module()
    in_maps = [{"xpack": xpacks[c], "wpack": wpacks[c]} for c in range(NCORES)]
    res = run_bass_kernel_spmd(nc, in_maps, list(range(NCORES)), trace=trace, **kw)
    total = res.results[0]["out"].astype(np.float32)
    for c in range(1, NCORES):
        total = total + res.results[c]["out"].astype(np.float32)
    full = np.ascontiguousarray(total.transpose(1, 0, 2))
    return full, res


def kernel(x, W):
    full, _ = run(x, W)
    return full



# revision 9
# speedup vs baseline: 1.2393x; 1.2393x over previous
"""Trainium2 Bass kernel for the ragged per-layer decoder stack.

out[b, i, a] = sum_{j<=i} sum_f x[b, j, f] * W[i, j, f, a]
  x: [256, 12, 2048] f32,  W: [12, 12, 2048, 768] f32 -> out: [256, 12, 768] f32

Sharding: W's d_features axis (F=2048) is split across the 8 NeuronCores
(256 features each). Each core contracts its feature slice against the
lower-triangular (j<=i) weight blocks and produces a full partial output
[12, 256, 768]; the host sums the 8 partials (the all-reduce) and
transposes back to [256, 12, 768].

Matmuls run in bf16 (hostside cast) with fp32 PSUM accumulation.
Weight DMAs are j-merged per (i, k-tile) and partition-major packed so
every partition row is one long contiguous run (>=4KB descriptors
saturate the HBM bus: measured 345 GB/s). Output DMAs go out on the ACT
HWDGE ring so their semaphore waits cannot head-of-line-block the W
stream on the SP ring. PSUM accumulation runs k-major so each group can
start as soon as its first k-block lands.
"""

import numpy as np
import ml_dtypes

import concourse.bass as bass
import concourse.tile as tile
from concourse import bacc, mybir
from concourse.bass_utils import run_bass_kernel_spmd

BF16 = ml_dtypes.bfloat16

# Problem shape (hardcoded per contract)
B = 256      # batch
L = 12       # layers
F = 2048     # d_features
A = 768      # d_activations
NCORES = 8
FC = F // NCORES      # feature slice per core = 256
P = 128               # partitions
NK = FC // P          # k-tiles per core slice = 2
NB = B // P           # batch tiles = 2
AC = 384              # activation chunk per matmul (2 chunks of 384 <= 512 PSUM)
NPAIR = sum(i + 1 for i in range(L)) * NK   # 156 weight tiles per core

_PAIRS = [(i, j) for i in range(L) for j in range(i + 1)]

# --- tuning knobs (affect build_module; set before first call) ---
WBUFS = 6         # W block pool slots (each sized [128, 12*768] bf16)
OBUFS = 4         # output tile pool slots
PSBUFS = 8        # PSUM pool slots (banks)
COPY_SPLIT = False  # alternate PSUM->SBUF copies between DVE and ACT
SKIP_MM = False     # diagnostic: drop matmuls+copies (DMA-only span)
SKIP_OUT = False    # diagnostic: drop copies + out-DMA
HWLOOP = True       # use tc.For_i for repeat>1 (bench only)
OBF16 = True        # write partial outputs as bf16 (host sums in fp32)
I_DESC = False      # process i in descending order (ascending measured faster)
KMAJOR = True       # accumulate k-major (j inner) so group starts on block k0
INTERLEAVE_AC = False  # interleave ac0/ac1 MMs sharing the stationary lhsT
ACSPLIT = False     # split A as 512+256 instead of 384+384
PSUM_DMA = False    # (unsupported: bass rejects DMA from PSUM)
PE_ONLY = False     # diagnostic: preload W for i<=IMAX once; loop MMs only
IMAX = L            # limit i range (diagnostics)
OUT_RING = "scalar"  # engine queue for out-DMA: scalar|sync|gpsimd
OUT_BATCH = False    # merge both bt out tiles into one [128, 2*768] DMA per i

# W block (i, k) tile offset in wpack: tiles [j=0..i] for fixed k
_WBASE = {}
_off = 0
for _i in range(L):
    for _k in range(NK):
        _WBASE[(_i, _k)] = _off
        _off += _i + 1
assert _off == NPAIR


def _emit_kernel(ctx, tc, xpack, wpack, out, repeat=1):
    nc = tc.nc
    xpool = ctx.enter_context(tc.tile_pool(name="xpool", bufs=1))
    wpool = ctx.enter_context(tc.tile_pool(name="wpool", bufs=WBUFS))
    opool = ctx.enter_context(tc.tile_pool(name="opool", bufs=OBUFS))
    pspool = ctx.enter_context(tc.tile_pool(name="pspool", bufs=PSBUFS, space="PSUM"))

    # x resident in SBUF for the whole kernel, one tile per k-slice:
    # xts[k][p, j*B + b] = x[b, j, c*FC + k*P + p]
    xts = []
    for k in range(NK):
        xt = xpool.tile([P, L * B], mybir.dt.bfloat16, tag=f"x{k}")
        nc.sync.dma_start(xt[:], xpack[:, k * L * B:(k + 1) * L * B])
        xts.append(xt)

    preloaded = None
    if PE_ONLY:
        # preload all W blocks for i < IMAX once; loop body has no W DMAs
        preloaded = {}
        for i in range(IMAX):
            n = i + 1
            for k in range(NK):
                wt = wpool.tile([P, n * A], mybir.dt.bfloat16,
                                name=f"wpre{i}_{k}", tag=f"wpre{i}_{k}", bufs=1)
                base = _WBASE[(i, k)] * A
                nc.sync.dma_start(wt[:], wpack[:, base:base + n * A])
                preloaded[(i, k)] = wt

    if repeat > 1 and HWLOOP:
        with tc.For_i(0, repeat, 1, hint_engines=(
                mybir.EngineType.PE, mybir.EngineType.SP)):
            _emit_body(tc, xts, wpack, out, wpool, opool, pspool, preloaded)
    else:
        for _ in range(repeat):
            _emit_body(tc, xts, wpack, out, wpool, opool, pspool, preloaded)


def _emit_body(tc, xts, wpack, out, wpool, opool, pspool, preloaded=None):
    nc = tc.nc
    iorder = range(L - 1, -1, -1) if I_DESC else range(L)
    for i in iorder:
        if i >= IMAX:
            continue
        n = i + 1
        # j-merged weight blocks, one per k-tile: [128, n*768] bf16.
        # wpack is partition-major, so each partition row is one
        # contiguous n*1536B run (>=4KB descriptors saturate the bus).
        wts = []
        for k in range(NK):
            if preloaded is not None:
                wts.append(preloaded[(i, k)])
                continue
            wt = wpool.tile([P, n * A], mybir.dt.bfloat16, tag="w")
            base = _WBASE[(i, k)] * A
            nc.sync.dma_start(wt[:], wpack[:, base:base + n * A])
            wts.append(wt)
        if KMAJOR:
            jks = [(j, k) for k in range(NK) for j in range(n)]
        else:
            jks = [(j, k) for j in range(n) for k in range(NK)]
        acs = [(0, 512), (512, 256)] if ACSPLIT else [(0, AC), (AC, AC)]
        obt = opool.tile([P, NB * A], mybir.dt.bfloat16, name="ob",
                         tag="ob") if OUT_BATCH else None
        for bt in range(NB):
            if SKIP_MM:
                continue
            pss = [pspool.tile([P, w], mybir.dt.float32, name=f"ps{ci}",
                               tag=f"ps{ci}", bufs=PSBUFS // 2)
                   for ci, (_, w) in enumerate(acs)]
            if INTERLEAVE_AC:
                for t, (j, k) in enumerate(jks):
                    lhsT = xts[k][:, j * B + bt * P:j * B + bt * P + P]
                    for ps, (off, w) in zip(pss, acs):
                        nc.tensor.matmul(
                            ps[:], lhsT,
                            wts[k][:, j * A + off:j * A + off + w],
                            start=(t == 0), stop=(t == len(jks) - 1),
                            skip_group_check=True,
                        )
            else:
                for ps, (off, w) in zip(pss, acs):
                    for t, (j, k) in enumerate(jks):
                        nc.tensor.matmul(
                            ps[:],
                            xts[k][:, j * B + bt * P:j * B + bt * P + P],
                            wts[k][:, j * A + off:j * A + off + w],
                            start=(t == 0), stop=(t == len(jks) - 1),
                        )
            if SKIP_OUT:
                continue
            if PSUM_DMA:
                for ps, (off, w) in zip(pss, acs):
                    nc.scalar.dma_start(
                        out[i, bt * P:(bt + 1) * P, off:off + w], ps[:])
                continue
            odt = mybir.dt.bfloat16 if OBF16 else mybir.dt.float32
            oeng = {"scalar": nc.scalar, "sync": nc.sync,
                    "gpsimd": nc.gpsimd}[OUT_RING]
            if OUT_BATCH:
                nc.vector.tensor_copy(
                    obt[:, bt * A:bt * A + acs[0][1]], pss[0][:])
                nc.vector.tensor_copy(
                    obt[:, bt * A + acs[0][1]:(bt + 1) * A], pss[1][:])
                if bt == NB - 1:
                    # out declared [L, P, NB*A] (batch-minor): one contiguous
                    # 393KB write per i
                    oeng.dma_start(out[i, :, :], obt[:])
                continue
            ot = opool.tile([P, A], odt)
            if COPY_SPLIT:
                nc.vector.tensor_copy(ot[:, 0:acs[0][1]], pss[0][:])
                nc.scalar.copy(ot[:, acs[0][1]:A], pss[1][:])
            else:
                nc.vector.tensor_copy(ot[:, 0:acs[0][1]], pss[0][:])
                nc.vector.tensor_copy(ot[:, acs[0][1]:A], pss[1][:])
            # out-DMA on the ACT HWDGE ring: its wait on the copy sem must
            # not head-of-line-block the W stream on the SP ring.
            oeng.dma_start(out[i, bt * P:(bt + 1) * P, :], ot[:])


_NC_CACHE = {}


def build_module(repeat=1):
    key = (repeat, WBUFS, OBUFS, PSBUFS, COPY_SPLIT, SKIP_MM, SKIP_OUT,
           HWLOOP, OBF16, I_DESC, KMAJOR, INTERLEAVE_AC, ACSPLIT, PSUM_DMA,
           PE_ONLY, IMAX, OUT_RING, OUT_BATCH)
    if key in _NC_CACHE:
        return _NC_CACHE[key]
    from contextlib import ExitStack
    nc = bacc.Bacc(
        "TRN2",
        target_bir_lowering=False,
        debug=False,
        enable_asserts=False,
        num_devices=NCORES,
    )
    xpack = nc.dram_tensor(
        "xpack", [P, NK * L * B], mybir.dt.bfloat16, kind="ExternalInput").ap()
    wpack = nc.dram_tensor(
        "wpack", [P, NPAIR * A], mybir.dt.bfloat16, kind="ExternalInput").ap()
    out = nc.dram_tensor(
        "out", [L, P, NB * A] if OUT_BATCH else [L, B, A],
        mybir.dt.bfloat16 if (OBF16 and not PSUM_DMA) else mybir.dt.float32,
        kind="ExternalOutput").ap()
    with tile.TileContext(nc) as tc:
        with ExitStack() as ctx:
            _emit_kernel(ctx, tc, xpack, wpack, out, repeat=repeat)
    nc.compile()
    _NC_CACHE[key] = nc
    return nc


def prep_inputs(x, W):
    """Build per-core packed inputs. Returns (xpacks[8], wpacks[8])."""
    # xpack[c][p, (k*L + j)*B + b] = x[b, j, c*FC + k*P + p]
    xb = np.asarray(x, dtype=BF16)                       # [256, 12, 2048]
    xr = xb.reshape(B, L, NCORES, NK, P).transpose(2, 4, 3, 1, 0)
    xpacks = np.ascontiguousarray(xr).reshape(NCORES, P, NK * L * B)

    # wpack[c]: partition-major; per (i, k) block occupies free columns
    # [_WBASE*A : (_WBASE+n)*A], j inner:
    #   wpack[c][p, (_WBASE[(i,k)] + j)*A + a] = W[i, j, c*FC + k*P + p, a]
    Ii = [i for i, j in _PAIRS]
    Jj = [j for i, j in _PAIRS]
    Wtri = np.asarray(W, dtype=BF16)[Ii, Jj]             # [78, 2048, 768]
    Wtri = Wtri.reshape(len(_PAIRS), NCORES, NK, P, A)   # [78, c, k, p, a]
    pidx = {}
    for t, (i, j) in enumerate(_PAIRS):
        pidx[(i, j)] = t
    sel_pair, sel_k = [], []
    for i in range(L):
        for k in range(NK):
            for j in range(i + 1):
                sel_pair.append(pidx[(i, j)])
                sel_k.append(k)
    Wp = Wtri[sel_pair, :, sel_k]                        # [156, c, 128, 768]
    Wp = np.ascontiguousarray(Wp.transpose(1, 2, 0, 3))  # [c, p, 156, a]
    wpacks = Wp.reshape(NCORES, P, NPAIR * A)
    return xpacks, wpacks


def run(x, W, trace=False, **kw):
    """Run the SPMD kernel; returns (full_output, BassKernelResults)."""
    x = np.asarray(x, dtype=np.float32)
    W = np.asarray(W, dtype=np.float32)
    xpacks, wpacks = prep_inputs(x, W)
    nc = build_module()
    in_maps = [{"xpack": xpacks[c], "wpack": wpacks[c]} for c in range(NCORES)]
    res = run_bass_kernel_spmd(nc, in_maps, list(range(NCORES)), trace=trace, **kw)
    total = res.results[0]["out"].astype(np.float32)
    for c in range(1, NCORES):
        total = total + res.results[c]["out"].astype(np.float32)
    if OUT_BATCH:
        total = total.reshape(L, P, NB, A).transpose(0, 2, 1, 3).reshape(L, B, A)
    full = np.ascontiguousarray(total.transpose(1, 0, 2))
    return full, res


def kernel(x, W):
    full, _ = run(x, W)
    return full



# revision 10
# speedup vs baseline: 2.0610x; 1.6631x over previous
"""Few-i sharded Trainium2 Bass kernel for the ragged per-layer decoder.

out[b, i, a] = sum_{j<=i} sum_f x[b, j, f] * W[i, j, f, a]
  x: [256, 12, 2048] f32,  W: [12, 12, 2048, 768] f32 -> out: [256, 12, 768]

Sharding: the 1248 weight blocks (i, k, j) with k in 0..15 (128-feature
slices), j <= i, are split into 32 single-i runs of sizes 52/48/40/16
(8 runs of each size; rows 16*(i+1) tile exactly).  Each core owns one
run of each size -> 156 blocks = equal W bytes, equal PE work, and only
FOUR partial-output rows per core (1.57MB written vs 4.7MB for
f-sharding).  Out-writes were measured to cost ~3.5x their bandwidth
share (they poison the W read stream), so minimizing write bytes is the
main lever; W streams at the full 358 GB/s/core HBM limit.

The program is identical on all cores: block t's x-stationary tile is
read from a per-core packed xw[:, t*256:(t+1)*256] (prep gathers the
right (j, k) slice per core), and W arrives as 39 chunk DMAs of 4
blocks.  The four PSUM groups (2 batch-tiles x 2 384-col halves) of a
segment stay open across its chunks (interleaved accumulation,
skip_group_check); segments alternate between two sets of 4 PSUM banks.
"""

import numpy as np
import ml_dtypes

import concourse.bass as bass
import concourse.tile as tile
from concourse import bacc, mybir
from concourse.bass_utils import run_bass_kernel_spmd

BF16 = ml_dtypes.bfloat16

B = 256      # batch
L = 12       # layers
F = 2048     # d_features
A = 768      # d_activations
NCORES = 8
P = 128      # partitions
NK = F // P  # 16 global k-tiles
NB = B // P  # 2 batch tiles
AC = 384     # activation chunk (2 x 384 = 768)
NBLK = 156   # blocks per core
CHUNK = 4    # blocks per W DMA

SEG = (52, 48, 40, 16)          # segment sizes, processing order
SEGB = [0, 52, 100, 140, 156]   # boundaries
NSEG = len(SEG)

# run assignment: c-th run of each size goes to core c (row = layer i)
RUNS = {
    52: [8, 8, 9, 9, 10, 10, 11, 11],
    48: [2, 3, 5, 5, 6, 6, 7, 11],
    40: [4, 4, 7, 7, 8, 9, 10, 11],
    16: [0, 1, 1, 3, 6, 9, 10, 10],
}
# SEG_ROW[c][s] = layer i that core c's segment s accumulates
SEG_ROW = [[RUNS[sz][c] for sz in SEG] for c in range(NCORES)]

# carve each row's block list into its runs (canonical order: size desc,
# then core asc) -> per-core block lists [(i, k, j)] in processing order
def _build_blocks():
    cursor = {i: 0 for i in range(L)}
    rows = {i: [(i, k, j) for k in range(NK) for j in range(i + 1)]
            for i in range(L)}
    core_seg_blocks = [[None] * NSEG for _ in range(NCORES)]
    for s, sz in enumerate(SEG):
        for c in range(NCORES):
            i = RUNS[sz][c]
            st = cursor[i]
            core_seg_blocks[c][s] = rows[i][st:st + sz]
            cursor[i] = st + sz
    for i in range(L):
        assert cursor[i] == len(rows[i])
    return [sum(segs, []) for segs in core_seg_blocks]

CORE_BLOCKS = _build_blocks()   # [8][156] of (i, k, j)

# --- tuning knobs ---
WBUFS = 8
OBUFS = 4
HWLOOP = True
OUT_RING = "scalar"
STAGGER = False     # staggered_reset on the For_i back-edge
OUT_MERGE = True    # one [128, 2*768] out-DMA per segment (batch-minor out)
SKIP_OUT = False    # diagnostic: drop copies + out-DMA
OUT_FP8 = False     # fp8e4m3 partials scaled by 0.5 (host rescales by 2)
# OUT_WIDE: partition-major out [P, NSEG*NB*A]; write bursts at group ends.
# 0=off, groups otherwise: 1=[(0,1,2,3)], 2=[(0,1),(2,3)], 3=[(0,1,2),(3,)]
OUT_WIDE = 0
_WIDE_GROUPS = {1: [(0, 1, 2, 3)], 2: [(0, 1), (2, 3)], 3: [(0, 1, 2), (3,)]}
# W_FP8: store the last W_FP8/4 of each segment's chunks as fp8e4m3
# (lhsT stays bf16).  0=off, 1=~quarter, 2=~half of chunks.
W_FP8 = 1


def _chunk_schedule(wfp8):
    """Uniform per-core chunk schedule: list of (is_fp8, blk_off_in_pack)
    per segment chunk, plus total bf16/fp8 block counts."""
    sched = []
    nb_off = 0
    f8_off = 0
    for s in range(NSEG):
        nch = SEG[s] // CHUNK
        nf8 = nch * wfp8 // 4
        for ci in range(nch):
            if ci < nch - nf8:
                sched.append((False, nb_off))
                nb_off += CHUNK
            else:
                sched.append((True, f8_off))
                f8_off += CHUNK
    return sched, nb_off, f8_off


def _emit_kernel(ctx, tc, xw, wpack, wpack8, out, repeat=1):
    nc = tc.nc
    xpool = ctx.enter_context(tc.tile_pool(name="xpool", bufs=1))
    wpool = ctx.enter_context(tc.tile_pool(name="wpool", bufs=WBUFS))
    opool = ctx.enter_context(tc.tile_pool(name="opool", bufs=OBUFS))
    pspool = ctx.enter_context(tc.tile_pool(name="pspool", bufs=2, space="PSUM"))

    # per-core packed x resident in SBUF: xt[p, t*B + b] = x[b, j_t, k_t*P+p]
    xt = xpool.tile([P, NBLK * B], mybir.dt.bfloat16, tag="xw")
    nc.sync.dma_start(xt[:], xw[:, :])

    if repeat > 1 and HWLOOP:
        with tc.For_i(0, repeat, 1, staggered_reset=STAGGER, hint_engines=(
                mybir.EngineType.PE, mybir.EngineType.SP)):
            _emit_body(tc, xt, wpack, wpack8, out, wpool, opool, pspool)
    else:
        for _ in range(repeat):
            _emit_body(tc, xt, wpack, wpack8, out, wpool, opool, pspool)


def _emit_body(tc, xt, wpack, wpack8, out, wpool, opool, pspool):
    nc = tc.nc
    sched, _, _ = _chunk_schedule(W_FP8)
    gchunk = 0
    oeng = {"scalar": nc.scalar, "sync": nc.sync,
            "gpsimd": nc.gpsimd}[OUT_RING]
    acs = [(0, AC), (AC, AC)]
    ow = None
    if OUT_WIDE:
        ow = opool.tile([P, NSEG * NB * A], mybir.dt.bfloat16, name="ow",
                        tag="ow")
    for s in range(NSEG):
        t0, t1 = SEGB[s], SEGB[s + 1]
        pss = {}
        for bt in range(NB):
            for ci in range(2):
                pss[(bt, ci)] = pspool.tile(
                    [P, AC], mybir.dt.float32, name=f"ps{bt}{ci}",
                    tag=f"ps{bt}{ci}")
        for c0 in range(t0, t1, CHUNK):
            is8, poff = sched[gchunk]
            gchunk += 1
            if is8:
                wt = wpool.tile([P, CHUNK * A], mybir.dt.float8e4,
                                name="w8", tag="w8")
                nc.sync.dma_start(wt[:], wpack8[:, poff * A:(poff + CHUNK) * A])
            else:
                wt = wpool.tile([P, CHUNK * A], mybir.dt.bfloat16,
                                name="w", tag="w")
                nc.sync.dma_start(wt[:], wpack[:, poff * A:(poff + CHUNK) * A])
            for t in range(c0, c0 + CHUNK):
                for bt in range(NB):
                    lhsT = xt[:, t * B + bt * P:t * B + bt * P + P]
                    for ci, (off, w) in enumerate(acs):
                        nc.tensor.matmul(
                            pss[(bt, ci)][:], lhsT,
                            wt[:, (t - c0) * A + off:(t - c0) * A + off + w],
                            start=(t == t0), stop=(t == t1 - 1),
                            skip_group_check=True,
                        )
        if SKIP_OUT:
            continue
        if OUT_WIDE:
            for bt in range(NB):
                nc.vector.tensor_copy(
                    ow[:, (s * NB + bt) * A:(s * NB + bt) * A + AC],
                    pss[(bt, 0)][:])
                nc.vector.tensor_copy(
                    ow[:, (s * NB + bt) * A + AC:(s * NB + bt + 1) * A],
                    pss[(bt, 1)][:])
            for grp in _WIDE_GROUPS[OUT_WIDE]:
                if s == grp[-1]:
                    c0, c1 = grp[0] * NB * A, (grp[-1] + 1) * NB * A
                    oeng.dma_start(out[:, c0:c1], ow[:, c0:c1])
            continue
        odt = mybir.dt.float8e4 if OUT_FP8 else mybir.dt.bfloat16

        def _cp(dst, ps):
            if OUT_FP8:
                nc.vector.tensor_scalar_mul(dst, ps, 0.5)
            else:
                nc.vector.tensor_copy(dst, ps)

        if OUT_MERGE:
            ot = opool.tile([P, NB * A], odt, name="om", tag="om")
            for bt in range(NB):
                _cp(ot[:, bt * A:bt * A + AC], pss[(bt, 0)][:])
                _cp(ot[:, bt * A + AC:(bt + 1) * A], pss[(bt, 1)][:])
            oeng.dma_start(out[s, :, :], ot[:])
        else:
            for bt in range(NB):
                ot = opool.tile([P, A], odt)
                _cp(ot[:, 0:AC], pss[(bt, 0)][:])
                _cp(ot[:, AC:A], pss[(bt, 1)][:])
                oeng.dma_start(out[s, bt * P:(bt + 1) * P, :], ot[:])


_NC_CACHE = {}


def build_module(repeat=1):
    key = (repeat, WBUFS, OBUFS, HWLOOP, OUT_RING, STAGGER, OUT_MERGE, CHUNK,
           SKIP_OUT, OUT_FP8, OUT_WIDE, W_FP8)
    if key in _NC_CACHE:
        return _NC_CACHE[key]
    from contextlib import ExitStack
    nc = bacc.Bacc(
        "TRN2",
        target_bir_lowering=False,
        debug=False,
        enable_asserts=False,
        num_devices=NCORES,
    )
    _, nbf, nf8 = _chunk_schedule(W_FP8)
    xw = nc.dram_tensor(
        "xpack", [P, NBLK * B], mybir.dt.bfloat16, kind="ExternalInput").ap()
    wpack = nc.dram_tensor(
        "wpack", [P, nbf * A], mybir.dt.bfloat16, kind="ExternalInput").ap()
    wpack8 = None
    if W_FP8:
        wpack8 = nc.dram_tensor(
            "wpack8", [P, nf8 * A], mybir.dt.float8e4,
            kind="ExternalInput").ap()
    if OUT_WIDE:
        oshape = [P, NSEG * NB * A]
    elif OUT_MERGE:
        oshape = [NSEG, P, NB * A]
    else:
        oshape = [NSEG, B, A]
    out = nc.dram_tensor(
        "out", oshape,
        mybir.dt.float8e4 if OUT_FP8 else mybir.dt.bfloat16,
        kind="ExternalOutput").ap()
    with tile.TileContext(nc) as tc:
        with ExitStack() as ctx:
            _emit_kernel(ctx, tc, xw, wpack, wpack8, out, repeat=repeat)
    nc.compile()
    _NC_CACHE[key] = nc
    return nc


def prep_inputs(x, W):
    """Build per-core packed inputs. Returns {name: [8, ...] array}."""
    F8 = ml_dtypes.float8_e4m3
    sched, nbf, nf8 = _chunk_schedule(W_FP8)
    # block index t -> (is_fp8, position within its pack), chunk-granular
    tmap = []
    for is8, poff in sched:
        for u in range(CHUNK):
            tmap.append((is8, poff + u))
    xb = np.asarray(x, dtype=BF16).reshape(B, L, NK, P)       # [b, j, k, p]
    Wb = np.asarray(W, dtype=BF16).reshape(L, L, NK, P, A)    # [i, j, k, p, a]
    xpacks = np.empty((NCORES, P, NBLK * B), dtype=BF16)
    wpacks = np.empty((NCORES, P, nbf * A), dtype=BF16)
    w8packs = np.empty((NCORES, P, nf8 * A), dtype=F8)
    for c in range(NCORES):
        Ii = np.array([b[0] for b in CORE_BLOCKS[c]])
        Kk = np.array([b[1] for b in CORE_BLOCKS[c]])
        Jj = np.array([b[2] for b in CORE_BLOCKS[c]])
        # xw[p, t*B + b] = x[b, j_t, k_t*P + p]
        xsel = xb[:, Jj, Kk]                    # [b, t, p]
        xpacks[c] = np.ascontiguousarray(
            xsel.transpose(2, 1, 0)).reshape(P, NBLK * B)
        # wpack[p, u*A + a] = W[i_t, j_t, k_t*P + p, a] for block t at
        # position u of its (bf16 | fp8) pack
        wsel = Wb[Ii, Jj, Kk].transpose(1, 0, 2)              # [p, t, a]
        wb = np.empty((P, nbf, A), dtype=BF16)
        w8 = np.empty((P, nf8, A), dtype=F8)
        for t, (is8, u) in enumerate(tmap):
            if is8:
                w8[:, u] = wsel[:, t].astype(F8)
            else:
                wb[:, u] = wsel[:, t]
        wpacks[c] = wb.reshape(P, nbf * A)
        w8packs[c] = w8.reshape(P, nf8 * A)
    d = {"xpack": xpacks, "wpack": wpacks}
    if W_FP8:
        d["wpack8"] = w8packs
    return d


def run(x, W, trace=False, **kw):
    """Run the SPMD kernel; returns (full_output, BassKernelResults)."""
    x = np.asarray(x, dtype=np.float32)
    W = np.asarray(W, dtype=np.float32)
    packs = prep_inputs(x, W)
    nc = build_module()
    in_maps = [{n: a[c] for n, a in packs.items()} for c in range(NCORES)]
    res = run_bass_kernel_spmd(nc, in_maps, list(range(NCORES)), trace=trace, **kw)
    full = np.zeros((L, B, A), dtype=np.float32)
    for c in range(NCORES):
        oc = res.results[c]["out"].astype(np.float32)
        if OUT_FP8:
            oc = oc * 2.0
        if OUT_WIDE:
            oc = oc.reshape(P, NSEG, NB, A).transpose(1, 2, 0, 3).reshape(
                NSEG, B, A)
        elif OUT_MERGE:
            oc = oc.reshape(NSEG, P, NB, A).transpose(0, 2, 1, 3).reshape(
                NSEG, B, A)
        for s in range(NSEG):
            full[SEG_ROW[c][s]] += oc[s]
    full = np.ascontiguousarray(full.transpose(1, 0, 2))
    return full, res


def kernel(x, W):
    full, _ = run(x, W)
    return full


# revision 11
# speedup vs baseline: 2.0837x; 1.0110x over previous
"""Few-i sharded Trainium2 Bass kernel for the ragged per-layer decoder.

out[b, i, a] = sum_{j<=i} sum_f x[b, j, f] * W[i, j, f, a]
  x: [256, 12, 2048] f32,  W: [12, 12, 2048, 768] f32 -> out: [256, 12, 768]

Sharding: the 1248 weight blocks (i, k, j) with k in 0..15 (128-feature
slices), j <= i, are split into 32 single-i runs of sizes 52/48/40/16
(8 runs of each size; rows 16*(i+1) tile exactly).  Each core owns one
run of each size -> 156 blocks = equal W bytes, equal PE work, and only
FOUR partial-output rows per core (1.57MB written vs 4.7MB for
f-sharding).  Out-writes were measured to cost ~3.5x their bandwidth
share (they poison the W read stream), so minimizing write bytes is the
main lever; W streams at the full 358 GB/s/core HBM limit.

The program is identical on all cores: block t's x-stationary tile is
read from a per-core packed xw[:, t*256:(t+1)*256] (prep gathers the
right (j, k) slice per core), and W arrives as 39 chunk DMAs of 4
blocks.  The four PSUM groups (2 batch-tiles x 2 384-col halves) of a
segment stay open across its chunks (interleaved accumulation,
skip_group_check); segments alternate between two sets of 4 PSUM banks.
"""

import numpy as np
import ml_dtypes

import concourse.bass as bass
import concourse.tile as tile
from concourse import bacc, mybir
from concourse.bass_utils import run_bass_kernel_spmd

BF16 = ml_dtypes.bfloat16

B = 256      # batch
L = 12       # layers
F = 2048     # d_features
A = 768      # d_activations
NCORES = 8
P = 128      # partitions
NK = F // P  # 16 global k-tiles
NB = B // P  # 2 batch tiles
AC = 384     # activation chunk (2 x 384 = 768)
NBLK = 156   # blocks per core
CHUNK = 4    # blocks per W DMA

SEG = (52, 48, 40, 16)          # segment sizes, processing order
SEGB = [0, 52, 100, 140, 156]   # boundaries
NSEG = len(SEG)

# run assignment: c-th run of each size goes to core c (row = layer i)
RUNS = {
    52: [8, 8, 9, 9, 10, 10, 11, 11],
    48: [2, 3, 5, 5, 6, 6, 7, 11],
    40: [4, 4, 7, 7, 8, 9, 10, 11],
    16: [0, 1, 1, 3, 6, 9, 10, 10],
}
# SEG_ROW[c][s] = layer i that core c's segment s accumulates
SEG_ROW = [[RUNS[sz][c] for sz in SEG] for c in range(NCORES)]

# carve each row's block list into its runs (canonical order: size desc,
# then core asc) -> per-core block lists [(i, k, j)] in processing order
def _build_blocks():
    cursor = {i: 0 for i in range(L)}
    rows = {i: [(i, k, j) for k in range(NK) for j in range(i + 1)]
            for i in range(L)}
    core_seg_blocks = [[None] * NSEG for _ in range(NCORES)]
    for s, sz in enumerate(SEG):
        for c in range(NCORES):
            i = RUNS[sz][c]
            st = cursor[i]
            core_seg_blocks[c][s] = rows[i][st:st + sz]
            cursor[i] = st + sz
    for i in range(L):
        assert cursor[i] == len(rows[i])
    return [sum(segs, []) for segs in core_seg_blocks]

CORE_BLOCKS = _build_blocks()   # [8][156] of (i, k, j)

# --- tuning knobs ---
WBUFS = 8
OBUFS = 4
HWLOOP = True
OUT_RING = "scalar"
STAGGER = False     # staggered_reset on the For_i back-edge
OUT_MERGE = True    # one [128, 2*768] out-DMA per segment (batch-minor out)
SKIP_OUT = False    # diagnostic: drop copies + out-DMA
OUT_FP8 = False     # fp8e4m3 partials scaled by 0.5 (host rescales by 2)
# OUT_WIDE: partition-major out [P, NSEG*NB*A]; write bursts at group ends.
# 0=off, groups otherwise: 1=[(0,1,2,3)], 2=[(0,1),(2,3)], 3=[(0,1,2),(3,)]
OUT_WIDE = 0
_WIDE_GROUPS = {1: [(0, 1, 2, 3)], 2: [(0, 1), (2, 3)], 3: [(0, 1, 2), (3,)]}
# W_FP8: store the last W_FP8/4 of each segment's chunks as fp8e4m3
# (lhsT stays bf16).  0=off, 1=~quarter, 2=~half of chunks.
W_FP8 = 2


def _chunk_schedule(wfp8):
    """Uniform per-core chunk schedule: list of (is_fp8, blk_off_in_pack)
    per segment chunk, plus total bf16/fp8 block counts."""
    sched = []
    nb_off = 0
    f8_off = 0
    for s in range(NSEG):
        nch = SEG[s] // CHUNK
        nf8 = nch * wfp8 // 4
        for ci in range(nch):
            if ci < nch - nf8:
                sched.append((False, nb_off))
                nb_off += CHUNK
            else:
                sched.append((True, f8_off))
                f8_off += CHUNK
    return sched, nb_off, f8_off


def _emit_kernel(ctx, tc, xw, wpack, wpack8, out, repeat=1):
    nc = tc.nc
    xpool = ctx.enter_context(tc.tile_pool(name="xpool", bufs=1))
    wpool = ctx.enter_context(tc.tile_pool(name="wpool", bufs=WBUFS))
    opool = ctx.enter_context(tc.tile_pool(name="opool", bufs=OBUFS))
    pspool = ctx.enter_context(tc.tile_pool(name="pspool", bufs=2, space="PSUM"))

    # per-core packed x resident in SBUF: xt[p, t*B + b] = x[b, j_t, k_t*P+p]
    xt = xpool.tile([P, NBLK * B], mybir.dt.bfloat16, tag="xw")
    nc.sync.dma_start(xt[:], xw[:, :])

    if repeat > 1 and HWLOOP:
        with tc.For_i(0, repeat, 1, staggered_reset=STAGGER, hint_engines=(
                mybir.EngineType.PE, mybir.EngineType.SP)):
            _emit_body(tc, xt, wpack, wpack8, out, wpool, opool, pspool)
    else:
        for _ in range(repeat):
            _emit_body(tc, xt, wpack, wpack8, out, wpool, opool, pspool)


def _emit_body(tc, xt, wpack, wpack8, out, wpool, opool, pspool):
    nc = tc.nc
    sched, _, _ = _chunk_schedule(W_FP8)
    gchunk = 0
    oeng = {"scalar": nc.scalar, "sync": nc.sync,
            "gpsimd": nc.gpsimd}[OUT_RING]
    acs = [(0, AC), (AC, AC)]
    ow = None
    if OUT_WIDE:
        ow = opool.tile([P, NSEG * NB * A], mybir.dt.bfloat16, name="ow",
                        tag="ow")
    for s in range(NSEG):
        t0, t1 = SEGB[s], SEGB[s + 1]
        pss = {}
        for bt in range(NB):
            for ci in range(2):
                pss[(bt, ci)] = pspool.tile(
                    [P, AC], mybir.dt.float32, name=f"ps{bt}{ci}",
                    tag=f"ps{bt}{ci}")
        for c0 in range(t0, t1, CHUNK):
            is8, poff = sched[gchunk]
            gchunk += 1
            if is8:
                wt = wpool.tile([P, CHUNK * A], mybir.dt.float8e4,
                                name="w8", tag="w8")
                nc.sync.dma_start(wt[:], wpack8[:, poff * A:(poff + CHUNK) * A])
            else:
                wt = wpool.tile([P, CHUNK * A], mybir.dt.bfloat16,
                                name="w", tag="w")
                nc.sync.dma_start(wt[:], wpack[:, poff * A:(poff + CHUNK) * A])
            for t in range(c0, c0 + CHUNK):
                for bt in range(NB):
                    lhsT = xt[:, t * B + bt * P:t * B + bt * P + P]
                    for ci, (off, w) in enumerate(acs):
                        nc.tensor.matmul(
                            pss[(bt, ci)][:], lhsT,
                            wt[:, (t - c0) * A + off:(t - c0) * A + off + w],
                            start=(t == t0), stop=(t == t1 - 1),
                            skip_group_check=True,
                        )
        if SKIP_OUT:
            continue
        if OUT_WIDE:
            for bt in range(NB):
                nc.vector.tensor_copy(
                    ow[:, (s * NB + bt) * A:(s * NB + bt) * A + AC],
                    pss[(bt, 0)][:])
                nc.vector.tensor_copy(
                    ow[:, (s * NB + bt) * A + AC:(s * NB + bt + 1) * A],
                    pss[(bt, 1)][:])
            for grp in _WIDE_GROUPS[OUT_WIDE]:
                if s == grp[-1]:
                    c0, c1 = grp[0] * NB * A, (grp[-1] + 1) * NB * A
                    oeng.dma_start(out[:, c0:c1], ow[:, c0:c1])
            continue
        odt = mybir.dt.float8e4 if OUT_FP8 else mybir.dt.bfloat16

        def _cp(dst, ps):
            if OUT_FP8:
                nc.vector.tensor_scalar_mul(dst, ps, 0.5)
            else:
                nc.vector.tensor_copy(dst, ps)

        if OUT_MERGE:
            ot = opool.tile([P, NB * A], odt, name="om", tag="om")
            for bt in range(NB):
                _cp(ot[:, bt * A:bt * A + AC], pss[(bt, 0)][:])
                _cp(ot[:, bt * A + AC:(bt + 1) * A], pss[(bt, 1)][:])
            oeng.dma_start(out[s, :, :], ot[:])
        else:
            for bt in range(NB):
                ot = opool.tile([P, A], odt)
                _cp(ot[:, 0:AC], pss[(bt, 0)][:])
                _cp(ot[:, AC:A], pss[(bt, 1)][:])
                oeng.dma_start(out[s, bt * P:(bt + 1) * P, :], ot[:])


_NC_CACHE = {}


def build_module(repeat=1):
    key = (repeat, WBUFS, OBUFS, HWLOOP, OUT_RING, STAGGER, OUT_MERGE, CHUNK,
           SKIP_OUT, OUT_FP8, OUT_WIDE, W_FP8)
    if key in _NC_CACHE:
        return _NC_CACHE[key]
    from contextlib import ExitStack
    nc = bacc.Bacc(
        "TRN2",
        target_bir_lowering=False,
        debug=False,
        enable_asserts=False,
        num_devices=NCORES,
    )
    _, nbf, nf8 = _chunk_schedule(W_FP8)
    xw = nc.dram_tensor(
        "xpack", [P, NBLK * B], mybir.dt.bfloat16, kind="ExternalInput").ap()
    wpack = nc.dram_tensor(
        "wpack", [P, nbf * A], mybir.dt.bfloat16, kind="ExternalInput").ap()
    wpack8 = None
    if W_FP8:
        wpack8 = nc.dram_tensor(
            "wpack8", [P, nf8 * A], mybir.dt.float8e4,
            kind="ExternalInput").ap()
    if OUT_WIDE:
        oshape = [P, NSEG * NB * A]
    elif OUT_MERGE:
        oshape = [NSEG, P, NB * A]
    else:
        oshape = [NSEG, B, A]
    out = nc.dram_tensor(
        "out", oshape,
        mybir.dt.float8e4 if OUT_FP8 else mybir.dt.bfloat16,
        kind="ExternalOutput").ap()
    with tile.TileContext(nc) as tc:
        with ExitStack() as ctx:
            _emit_kernel(ctx, tc, xw, wpack, wpack8, out, repeat=repeat)
    nc.compile()
    _NC_CACHE[key] = nc
    return nc


def prep_inputs(x, W):
    """Build per-core packed inputs. Returns {name: [8, ...] array}."""
    F8 = ml_dtypes.float8_e4m3
    sched, nbf, nf8 = _chunk_schedule(W_FP8)
    # block index t -> (is_fp8, position within its pack), chunk-granular
    tmap = []
    for is8, poff in sched:
        for u in range(CHUNK):
            tmap.append((is8, poff + u))
    xb = np.asarray(x, dtype=BF16).reshape(B, L, NK, P)       # [b, j, k, p]
    Wb = np.asarray(W, dtype=BF16).reshape(L, L, NK, P, A)    # [i, j, k, p, a]
    xpacks = np.empty((NCORES, P, NBLK * B), dtype=BF16)
    wpacks = np.empty((NCORES, P, nbf * A), dtype=BF16)
    w8packs = np.empty((NCORES, P, nf8 * A), dtype=F8)
    for c in range(NCORES):
        Ii = np.array([b[0] for b in CORE_BLOCKS[c]])
        Kk = np.array([b[1] for b in CORE_BLOCKS[c]])
        Jj = np.array([b[2] for b in CORE_BLOCKS[c]])
        # xw[p, t*B + b] = x[b, j_t, k_t*P + p]
        xsel = xb[:, Jj, Kk]                    # [b, t, p]
        xpacks[c] = np.ascontiguousarray(
            xsel.transpose(2, 1, 0)).reshape(P, NBLK * B)
        # wpack[p, u*A + a] = W[i_t, j_t, k_t*P + p, a] for block t at
        # position u of its (bf16 | fp8) pack
        wsel = Wb[Ii, Jj, Kk].transpose(1, 0, 2)              # [p, t, a]
        wb = np.empty((P, nbf, A), dtype=BF16)
        w8 = np.empty((P, nf8, A), dtype=F8)
        for t, (is8, u) in enumerate(tmap):
            if is8:
                w8[:, u] = wsel[:, t].astype(F8)
            else:
                wb[:, u] = wsel[:, t]
        wpacks[c] = wb.reshape(P, nbf * A)
        w8packs[c] = w8.reshape(P, nf8 * A)
    d = {"xpack": xpacks, "wpack": wpacks}
    if W_FP8:
        d["wpack8"] = w8packs
    return d


def run(x, W, trace=False, **kw):
    """Run the SPMD kernel; returns (full_output, BassKernelResults)."""
    x = np.asarray(x, dtype=np.float32)
    W = np.asarray(W, dtype=np.float32)
    packs = prep_inputs(x, W)
    nc = build_module()
    in_maps = [{n: a[c] for n, a in packs.items()} for c in range(NCORES)]
    res = run_bass_kernel_spmd(nc, in_maps, list(range(NCORES)), trace=trace, **kw)
    full = np.zeros((L, B, A), dtype=np.float32)
    for c in range(NCORES):
        oc = res.results[c]["out"].astype(np.float32)
        if OUT_FP8:
            oc = oc * 2.0
        if OUT_WIDE:
            oc = oc.reshape(P, NSEG, NB, A).transpose(1, 2, 0, 3).reshape(
                NSEG, B, A)
        elif OUT_MERGE:
            oc = oc.reshape(NSEG, P, NB, A).transpose(0, 2, 1, 3).reshape(
                NSEG, B, A)
        for s in range(NSEG):
            full[SEG_ROW[c][s]] += oc[s]
    full = np.ascontiguousarray(full.transpose(1, 0, 2))
    return full, res


def kernel(x, W):
    full, _ = run(x, W)
    return full
